# revision 10
# baseline (speedup 1.0000x reference)
"""Trainium2 Bass kernel for nn_AttentiveEncoder_73469710566059.

Reference computation (N=50000, D=1024, 4 layers of diagonal scale):
    y = x
    for i in range(4):
        y = y * w[i]          # elementwise scale along D
        if i != 3: y = relu(y)

Math fold: after layer 0, y0 = relu(x*w0) >= 0, so every later
relu(y * w_i) == y * max(w_i, 0).  Hence

    y = relu(x * w[0]) * c,      c = max(w[1],0) * max(w[2],0) * w[3]

with c a (D,) vector computed on the host (w is tiny).  On-device work is
2 DVE ops/element (tensor_mul + fused (max 0, mult) scalar_tensor_tensor).
When w[0] == 1 and c == 1 elementwise (e.g. the module's init state,
w = ones), the scales are identities and y == relu(x) bitwise, so a
specialized relu-only kernel runs instead: 1 DVE op/element and no
constant-tile DMA traffic at all.

The problem is memory-bound: 25.6 MB in + 25.6 MB out per core; one
NeuronCore's 16 SDMA engines sustain ~435 GB/s combined (SBUF AXI
fabric), so the floor is ~118 us of streaming + ramp/drain.

Sharding: data-parallel over N.  50000 rows / 8 cores = 6250 rows/core;
each core's (6250, 1024) shard is viewed flat as [128 partitions, 50000]
(6250*1024/128 == 50000, no padding).  In that view, element (p, j) has
d-coordinate (848*p + j) mod 1024 (50000 mod 1024 == 848), so the host
passes per-partition phase-rotated broadcast tiles of w0 and c.

DMA ring usage: x loads stream on the sync engine's HWDGE ring, stores
(and const tiles) on the scalar engine's ring — HWDGE rings are FIFO per
issuing engine, so a store waiting on compute must not block the load
stream.  The last few stores are issued on the sync ring (after its last
load) so both rings share the store drain at the tail.
"""

import numpy as np

N = 50000
D = 1024
N_CORES = 8
ROWS = N // N_CORES            # 6250 rows per core
FLAT = ROWS * D // 128         # 50000 elements per partition
PHASE = FLAT % D               # 848
CHUNK = 8192                   # free-dim elements per tile; multiple of 1024 so
                               # the phase-rotated w pattern stays aligned
N_BUFS = 5
STORE_DELAY = 3                # emit store k after load k+3: a store's sem wait
                               # then never starves its ring's sequencer

_STATE = {}


def _chunks():
    out = []
    j = 0
    while j < FLAT:
        cw = min(CHUNK, FLAT - j)
        out.append((j, cw))
        j += cw
    return out


def _build_bass(identity):
    from concourse import bacc, tile
    import concourse.mybir as mybir

    f32 = mybir.dt.float32
    # Bacc (not raw Bass): its compile() pass splits multi-wait sync infos
    # (TRN2 allows at most one wait per instruction) via event semaphores.
    nc = bacc.Bacc(None)
    x_in = nc.declare_dram_parameter("x", [128, FLAT], f32, isOutput=False)
    if not identity:
        w0_in = nc.declare_dram_parameter("w0t", [128, CHUNK], f32, isOutput=False)
        c_in = nc.declare_dram_parameter("ct", [128, CHUNK], f32, isOutput=False)
    y_out = nc.declare_dram_parameter("y", [128, FLAT], f32, isOutput=True)

    chunks = _chunks()
    n_chunks = len(chunks)

    with tile.TileContext(nc) as tc:
        with (
            tc.tile_pool(name="consts", bufs=1) as cpool,
            tc.tile_pool(name="work", bufs=N_BUFS) as wpool,
        ):
            if not identity:
                w0 = cpool.tile([128, CHUNK], f32, tag="w0")
                ct = cpool.tile([128, CHUNK], f32, tag="ct")
                nc.scalar.dma_start(out=w0[:], in_=w0_in[:])
                nc.sync.dma_start(out=ct[:], in_=c_in[:])

            # Symmetric ring interleave: load k on ring k%2, store k on the
            # other ring, so both HWDGE rings stream exactly half the bytes
            # and drain together at the tail.
            rings = [nc.sync, nc.scalar]
            tiles = {}

            def emit_store(k):
                j, cw = chunks[k]
                t = tiles.pop(k)
                rings[(k + 1) % 2].dma_start(
                    out=y_out[:, j : j + cw], in_=t[:, :cw]
                )

            for k, (j, cw) in enumerate(chunks):
                t = wpool.tile([128, CHUNK], f32, tag="x")
                tiles[k] = t
                rings[k % 2].dma_start(out=t[:, :cw], in_=x_in[:, j : j + cw])
                if identity:
                    nc.vector.tensor_scalar_max(t[:, :cw], t[:, :cw], 0.0)
                else:
                    nc.vector.tensor_mul(t[:, :cw], t[:, :cw], w0[:, :cw])
                    nc.vector.scalar_tensor_tensor(
                        t[:, :cw],
                        t[:, :cw],
                        0.0,
                        ct[:, :cw],
                        op0=mybir.AluOpType.max,
                        op1=mybir.AluOpType.mult,
                    )
                if k >= STORE_DELAY:
                    emit_store(k - STORE_DELAY)
            for k in range(max(0, n_chunks - STORE_DELAY), n_chunks):
                emit_store(k)
    nc.finalize()
    return nc


def _get_nc(identity):
    key = ("nc", bool(identity))
    if key not in _STATE:
        _STATE[key] = _build_bass(identity)
    return _STATE[key]


def _fold_w(w):
    """(w0, c) such that the network is y = relu(x*w0) * c."""
    w = np.asarray(w, dtype=np.float32)
    n_layers = w.shape[0]
    c = w[n_layers - 1].copy()
    for i in range(n_layers - 2, 0, -1):
        c = np.maximum(w[i], 0.0) * c
    return w[0], c


def _host_tiles(w0, c):
    """Phase-rotated broadcast tiles for w0 and c."""
    p = np.arange(128)[:, None]
    j = np.arange(CHUNK)[None, :]
    idx = (PHASE * p + j) % D
    return np.ascontiguousarray(w0[idx]), np.ascontiguousarray(c[idx])


def run_spmd(x, w, trace=False, **spmd_kwargs):
    """Shard, run on 8 cores, gather.  Returns (y_full, BassKernelResults)."""
    from concourse.bass_utils import run_bass_kernel_spmd

    x = np.asarray(x)
    assert x.shape == (N, D), x.shape
    w0, c = _fold_w(w)
    identity = bool(np.all(w0 == 1.0) and np.all(c == 1.0))
    xs = np.ascontiguousarray(x).reshape(N_CORES, 128, FLAT)
    if identity:
        in_maps = [{"x": xs[i]} for i in range(N_CORES)]
    else:
        w0t, ct = _host_tiles(w0, c)
        in_maps = [{"x": xs[i], "w0t": w0t, "ct": ct} for i in range(N_CORES)]
    res = run_bass_kernel_spmd(
        _get_nc(identity), in_maps, list(range(N_CORES)), trace=trace, **spmd_kwargs
    )
    y = np.stack([res.results[i]["y"] for i in range(N_CORES)], axis=0)
    return y.reshape(N, D).astype(np.float32, copy=False), res


def kernel(x, w):
    y, _ = run_spmd(x, w, trace=False)
    return y


# revision 11
# speedup vs baseline: 1.1193x; 1.1193x over previous
"""Trainium2 Bass kernel for nn_AttentiveEncoder_73469710566059.

Reference computation (N=50000, D=1024, 4 layers of diagonal scale):
    y = x
    for i in range(4):
        y = y * w[i]          # elementwise scale along D
        if i != 3: y = relu(y)

Math fold: after layer 0, y0 = relu(x*w0) >= 0, so every later
relu(y * w_i) == y * max(w_i, 0).  Hence

    y = relu(x * w[0]) * c,      c = max(w[1],0) * max(w[2],0) * w[3]

with c a (D,) vector computed on the host (w is tiny).  On-device work is
2 DVE ops/element (tensor_mul + fused (max 0, mult) scalar_tensor_tensor).
When w[0] == 1 and c == 1 elementwise (e.g. the module's init state,
w = ones), the scales are identities and y == relu(x) bitwise, so a
specialized relu-only kernel runs instead: 1 DVE op/element and no
constant-tile DMA traffic at all.

The problem is memory-bound: 25.6 MB in + 25.6 MB out per core; one
NeuronCore's 16 SDMA engines sustain ~435 GB/s combined (SBUF AXI
fabric), so the floor is ~118 us of streaming + ramp/drain.

Sharding: data-parallel over N.  50000 rows / 8 cores = 6250 rows/core;
each core's (6250, 1024) shard is viewed flat as [128 partitions, 50000]
(6250*1024/128 == 50000, no padding).  In that view, element (p, j) has
d-coordinate (848*p + j) mod 1024 (50000 mod 1024 == 848), so the host
passes per-partition phase-rotated broadcast tiles of w0 and c.

DMA ring usage: x loads stream on the sync engine's HWDGE ring, stores
(and const tiles) on the scalar engine's ring — HWDGE rings are FIFO per
issuing engine, so a store waiting on compute must not block the load
stream.  The last few stores are issued on the sync ring (after its last
load) so both rings share the store drain at the tail.
"""

import numpy as np

N = 50000
D = 1024
N_CORES = 8
ROWS = N // N_CORES            # 6250 rows per core
FLAT = ROWS * D // 128         # 50000 elements per partition
PHASE = FLAT % D               # 848
CHUNK = 4096                   # free-dim elements per tile; multiple of 1024 so
                               # the phase-rotated w pattern stays aligned
N_BUFS = 10
STORE_DELAY = 3                # emit store k after load k+3: a store's sem wait
                               # then never starves its ring's sequencer

_STATE = {}


def _chunks():
    out = []
    j = 0
    while j < FLAT:
        cw = min(CHUNK, FLAT - j)
        out.append((j, cw))
        j += cw
    return out


def _build_bass(identity):
    from concourse import bacc, tile
    import concourse.mybir as mybir

    f32 = mybir.dt.float32
    # Bacc (not raw Bass): its compile() pass splits multi-wait sync infos
    # (TRN2 allows at most one wait per instruction) via event semaphores.
    nc = bacc.Bacc(None)
    x_in = nc.declare_dram_parameter("x", [128, FLAT], f32, isOutput=False)
    if not identity:
        w0_in = nc.declare_dram_parameter("w0t", [128, CHUNK], f32, isOutput=False)
        c_in = nc.declare_dram_parameter("ct", [128, CHUNK], f32, isOutput=False)
    y_out = nc.declare_dram_parameter("y", [128, FLAT], f32, isOutput=True)

    chunks = _chunks()
    n_chunks = len(chunks)

    with tile.TileContext(nc) as tc:
        with (
            tc.tile_pool(name="consts", bufs=1) as cpool,
            tc.tile_pool(name="work", bufs=N_BUFS) as wpool,
        ):
            if not identity:
                w0 = cpool.tile([128, CHUNK], f32, tag="w0")
                ct = cpool.tile([128, CHUNK], f32, tag="ct")
                nc.scalar.dma_start(out=w0[:], in_=w0_in[:])
                nc.sync.dma_start(out=ct[:], in_=c_in[:])

            # Symmetric ring interleave: load k on ring k%2, store k on the
            # other ring, so both HWDGE rings stream exactly half the bytes
            # and drain together at the tail.
            rings = [nc.sync, nc.scalar]
            tiles = {}

            def emit_store(k):
                j, cw = chunks[k]
                t = tiles.pop(k)
                rings[(k + 1) % 2].dma_start(
                    out=y_out[:, j : j + cw], in_=t[:, :cw]
                )

            for k, (j, cw) in enumerate(chunks):
                t = wpool.tile([128, CHUNK], f32, tag="x")
                tiles[k] = t
                rings[k % 2].dma_start(out=t[:, :cw], in_=x_in[:, j : j + cw])
                if identity:
                    nc.vector.tensor_scalar_max(t[:, :cw], t[:, :cw], 0.0)
                else:
                    nc.vector.tensor_mul(t[:, :cw], t[:, :cw], w0[:, :cw])
                    nc.vector.scalar_tensor_tensor(
                        t[:, :cw],
                        t[:, :cw],
                        0.0,
                        ct[:, :cw],
                        op0=mybir.AluOpType.max,
                        op1=mybir.AluOpType.mult,
                    )
                if k >= STORE_DELAY:
                    emit_store(k - STORE_DELAY)
            for k in range(max(0, n_chunks - STORE_DELAY), n_chunks):
                emit_store(k)
    nc.finalize()
    return nc


def _get_nc(identity):
    key = ("nc", bool(identity))
    if key not in _STATE:
        _STATE[key] = _build_bass(identity)
    return _STATE[key]


def _fold_w(w):
    """(w0, c) such that the network is y = relu(x*w0) * c."""
    w = np.asarray(w, dtype=np.float32)
    n_layers = w.shape[0]
    c = w[n_layers - 1].copy()
    for i in range(n_layers - 2, 0, -1):
        c = np.maximum(w[i], 0.0) * c
    return w[0], c


def _host_tiles(w0, c):
    """Phase-rotated broadcast tiles for w0 and c."""
    p = np.arange(128)[:, None]
    j = np.arange(CHUNK)[None, :]
    idx = (PHASE * p + j) % D
    return np.ascontiguousarray(w0[idx]), np.ascontiguousarray(c[idx])


def run_spmd(x, w, trace=False, **spmd_kwargs):
    """Shard, run on 8 cores, gather.  Returns (y_full, BassKernelResults)."""
    from concourse.bass_utils import run_bass_kernel_spmd

    x = np.asarray(x)
    assert x.shape == (N, D), x.shape
    w0, c = _fold_w(w)
    identity = bool(np.all(w0 == 1.0) and np.all(c == 1.0))
    xs = np.ascontiguousarray(x).reshape(N_CORES, 128, FLAT)
    if identity:
        in_maps = [{"x": xs[i]} for i in range(N_CORES)]
    else:
        w0t, ct = _host_tiles(w0, c)
        in_maps = [{"x": xs[i], "w0t": w0t, "ct": ct} for i in range(N_CORES)]
    res = run_bass_kernel_spmd(
        _get_nc(identity), in_maps, list(range(N_CORES)), trace=trace, **spmd_kwargs
    )
    y = np.stack([res.results[i]["y"] for i in range(N_CORES)], axis=0)
    return y.reshape(N, D).astype(np.float32, copy=False), res


def kernel(x, w):
    y, _ = run_spmd(x, w, trace=False)
    return y
